# revision 20
# baseline (speedup 1.0000x reference)
"""Distributed Bass kernel for nn_Attention (B=2, S=2048, HID=2048, H=32, KVH=8, D=64).

Sharding (8 NeuronCores, uniform SPMD):
  - Head-parallel attention: core c owns kv-head c + its 4 GQA query heads.
    x replicated (host-transposed, tile-packed bf16); per-core Q^T [256,4096],
    K^T [64,4096] (+RoPE, 1/sqrt(D) folded into the Q trig tables) and
    V [tok,64] per key block with a shared ones-column for the softmax
    denominator.
  - Phase-1 chunks and attention are software-pipelined with a one-chunk
    lag: chunk N's projection matmuls are interleaved between chunk N-1's
    attention units so the PE stays dense while ACT (exp — the attention
    bottleneck) drains, and the HAM clock gate stays released.
  - Attention processes a head PAIR per unit: the two heads' S^T matmuls
    sit on disjoint PE row groups back-to-back, one exp covers both, and
    the AV matmuls share the stationary V block.
  - Causal handling: S^T/AV matmuls are ragged on diagonal key-blocks
    (fully-masked query ranges skipped); only the 128x128 triangle is
    multiplied by a mask.
  - One AllToAll per head-pair re-shards attn^T to token-parallel; raw f32
    denominators ride in the same buffer as four bf16-bitcast rows.
    The final chunk runs pair 1 first so its collective hides under
    pair 0's attention tail.
  - Phase 2 (token-parallel): out rows = attn^T.T @ wo, wo prefetched
    during attention.

All large inputs are host-prepacked so each transfer is a contiguous
block (full-rate DMA descriptors).  PE warmup matmuls at t=0 release the
HAM clock gate before real work arrives.
"""

import numpy as np
import ml_dtypes

import concourse.bass as bass
import concourse.mybir as mybir
import concourse.tile as tile
from concourse import bacc
from concourse.bass_utils import run_bass_kernel_spmd

BF16 = ml_dtypes.bfloat16
F32 = np.float32

B, S, HID = 2, 2048, 2048
H, KVH, D = 32, 8, 64
NC = 8
T = B * S              # 4096 flat tokens
TL = T // NC           # 512 tokens per core (phase-2 output rows)
LH = H // NC           # 4 local q-heads
KB = 128               # key block
TC = 512               # phase-1 token chunk / attention query chunk
NTC = T // TC          # 8 chunks

_CACHE = {}


def _build():
    fp32 = mybir.dt.float32
    bf16 = mybir.dt.bfloat16

    nc = bacc.Bacc("TRN2", target_bir_lowering=False, debug=False, num_devices=NC)

    xTb = nc.dram_tensor("xTb", [NTC, 128, 16 * TC], bf16, kind="ExternalInput")
    trig_d = nc.dram_tensor("trig", [NTC, 128, 3 * TC], fp32, kind="ExternalInput")
    wqkv_d = nc.dram_tensor("wqkv", [128, 16 * 384], bf16, kind="ExternalInput")
    wob_d = nc.dram_tensor("wob", [4, 128, 16 * TC], bf16, kind="ExternalInput")
    tri_d = nc.dram_tensor("tri", [128, 4 * KB], bf16, kind="ExternalInput")
    out_d = nc.dram_tensor("out", [TL, HID], fp32, kind="ExternalOutput")

    with tile.TileContext(nc) as tc:
        with (
            tc.tile_pool(name="persist", bufs=1) as persist,
            tc.tile_pool(name="xs", bufs=2) as xs,
            tc.tile_pool(name="trg", bufs=2) as trg,
            tc.tile_pool(name="work", bufs=2) as work,
            tc.tile_pool(name="expool", bufs=4) as expool,
            tc.tile_pool(name="aop", bufs=1) as aopool,
            tc.tile_pool(name="projq", bufs=1, space="PSUM") as projq,
            tc.tile_pool(name="pss", bufs=2, space="PSUM") as pss,
            tc.tile_pool(name="po", bufs=2, space="PSUM") as po,
            tc.tile_pool(name="dram", bufs=1, space="DRAM") as dram,
        ):
            # ---- persistent tiles ----
            qT = [persist.tile([128, T], bf16, tag=f"qT{t}", name=f"qT{t}")
                  for t in range(2)]
            k2 = persist.tile([128, T], bf16, tag="k2", name="k2")
            vatt = persist.tile([128, (T // KB) * 65], bf16, tag="vatt", name="vatt")
            attnT = [persist.tile([128, T], bf16, tag=f"attnT{t}", name=f"attnT{t}")
                     for t in range(2)]
            tri = persist.tile([128, 4 * KB], bf16, tag="tri", name="tri")
            wqkv = persist.tile([128, 16 * 384], bf16, tag="wqkv", name="wqkv")
            wop = [persist.tile([128, 16 * TC], bf16, tag=f"wop{t}", name=f"wop{t}")
                   for t in range(2)]
            ident = persist.tile([128, 128], bf16, tag="ident", name="ident")

            nc.sync.dma_start(wqkv[:], wqkv_d[:])
            nc.sync.dma_start(tri[:], tri_d[:])
            from concourse.masks import make_identity
            make_identity(nc, ident[:])
            # ones-columns of vatt (65th col of each of the 32 key blocks)
            nc.gpsimd.memset(
                vatt[:, :].rearrange("p (n w) -> p n w", n=T // KB)[:, :, 64:65],
                1.0)

            # ---- PE warmup: release the HAM clock gate during input DMA ----
            wps = po.tile([128, TC], fp32, tag="po", name="warm")
            for _ in range(300):
                nc.tensor.matmul(wps[:, 0:128], ident[:], ident[:],
                                 start=True, stop=True)

            # ---- collectives staging (DRAM) ----
            # rows 0:128 = attn^T, rows 128:132 = two raw f32 denominator
            # rows bit-cast to four bf16 rows (AllToAll is bypass).
            a2a_in = [dram.tile([NC, 132, TL], bf16, tag=f"a2a_in{t}",
                                name=f"a2a_in{t}") for t in range(2)]
            a2a_out = [dram.tile([NC, 132, TL], bf16, tag=f"a2a_out{t}",
                                 name=f"a2a_out{t}") for t in range(2)]

            def rope(out_ap, ps, ct, st, npart):
                """out = ps*ct + swap32(ps)*st (st carries the rotate-half sign)."""
                t1 = work.tile([128, TC], fp32, tag="rope_t1", name="t1")
                t2 = work.tile([128, TC], fp32, tag="rope_t2", name="t2")
                nc.vector.tensor_mul(t1[:npart, :], ps[:npart, :], ct[:npart, :])
                for base in range(0, npart, 64):
                    a, b2 = base, base + 32
                    nc.vector.tensor_mul(t2[a:a + 32, :], ps[b2:b2 + 32, :],
                                         st[a:a + 32, :])
                    nc.vector.tensor_mul(t2[b2:b2 + 32, :], ps[a:a + 32, :],
                                         st[b2:b2 + 32, :])
                nc.vector.tensor_add(out_ap, t1[:npart, :], t2[:npart, :])

            def attn_pair_steps(pair, b, cq):
                """Causal attention for one head pair over a 512-query chunk,
                as a list of per-key-block emission steps (so projection
                matmul pieces can interleave and keep ACT fed).  Each step k
                emits S^T(k)+exp(k)+mask(k) and AV(k-1); a final step emits
                the last AV and the drains."""
                qtile = qT[pair]
                nkb = 4 * (cq + 1)
                qs = S * b + TC * cq
                st = {}

                def stile(kbi):
                    d = kbi - (nkb - 4)   # >=0 on diagonal key-blocks
                    qo = 128 * d if d >= 0 else 0
                    kpos = S * b + KB * kbi
                    psS = pss.tile([128, 2 * TC], fp32, tag="pss",
                                   name=f"psS{pair}_{b}_{cq}_{kbi}")
                    for h in range(2):
                        nc.tensor.matmul(
                            psS[:, TC * h + qo:TC * (h + 1)],
                            k2[64 * h:64 * h + 64, kpos:kpos + KB],
                            qtile[64 * h:64 * h + 64, qs + qo:qs + TC],
                            start=True, stop=True)
                    ex = expool.tile([128, 2 * TC + 3 * KB], bf16, tag="ex",
                                     name="ex")
                    nc.scalar.activation(ex[:, 0:2 * TC], psS[:],
                                         mybir.ActivationFunctionType.Exp)
                    if d >= 0:
                        # 128x128 triangle mask (both head segments, one
                        # strided op) on gpsimd — keeps it off the DVE FIFO
                        exm = ex[:, qo:qo + 2 * TC].rearrange(
                            "p (n w) -> p n w", n=2)[:, :, 0:KB]
                        nc.gpsimd.tensor_mul(
                            exm, exm,
                            tri[:, 0:2 * KB].rearrange("p (n w) -> p n w", n=2))
                    st[kbi] = (ex, qo)

                def av(kbi):
                    ex, qo = st.pop(kbi)
                    vb = vatt[:, 65 * (16 * b + kbi):65 * (16 * b + kbi) + 65]
                    for h in range(2):
                        nc.tensor.matmul(
                            st["psO"][h][0:65, qo:TC], vb,
                            ex[:, TC * h + qo:TC * (h + 1)],
                            start=(kbi == 0), stop=(kbi == nkb - 1),
                            skip_group_check=True)

                def first():
                    st["psO"] = [po.tile([128, TC], fp32, tag="po",
                                         name=f"psO{pair}{h}_{b}_{cq}")
                                 for h in range(2)]
                    stile(0)

                def mid(kbi):
                    stile(kbi)
                    av(kbi - 1)

                def last():
                    av(nkb - 1)
                    jj = 4 * b + cq
                    psO = st.pop("psO")
                    for h in range(2):
                        nc.vector.tensor_copy(
                            attnT[pair][64 * h:64 * h + 64, qs:qs + TC],
                            psO[h][0:D, :])
                        ds = work.tile([1, TC], fp32, tag="ds", name="ds")
                        nc.vector.tensor_copy(ds[:], psO[h][D:D + 1, :])
                        nc.gpsimd.dma_start(
                            a2a_in[pair][jj, 128 + 2 * h:130 + 2 * h, :],
                            ds[:].bitcast(bf16))
                    nc.sync.dma_start(a2a_in[pair][jj, 0:128, :],
                                      attnT[pair][:, TC * jj:TC * (jj + 1)])

                return ([first] + [(lambda k: lambda: mid(k))(k)
                                   for k in range(1, nkb)] + [last])

            def attn_pair(pair, b, cq):
                for s in attn_pair_steps(pair, b, cq):
                    s()

            def fire_a2a(t):
                nc.gpsimd.collective_compute(
                    "AllToAll", mybir.AluOpType.bypass,
                    replica_groups=[list(range(NC))],
                    ins=[a2a_in[t].opt()], outs=[a2a_out[t].opt()])

            # ========== pipelined phase 1 + attention (one-chunk lag) ==========
            # Chunk N's projection matmuls are emitted in 8-matmul pieces
            # interleaved between chunk N-1's pair-0 attention steps, so the
            # PE queue never has a long projection block starving ACT.
            def build_pieces(tc8, xbig, trgt):
                tsl = slice(TC * tc8, TC * (tc8 + 1))
                hold = {}

                def q_piece(qt, half):
                    def f():
                        if "q" not in hold:
                            hold["q"] = projq.tile([128, 2 * TC], fp32,
                                                   tag="pq", name=f"pq{tc8}")
                        for k in range(8 * half, 8 * half + 8):
                            nc.tensor.matmul(
                                hold["q"][:, TC * qt:TC * (qt + 1)],
                                wqkv[:, 384 * k + 128 * qt:
                                     384 * k + 128 * (qt + 1)],
                                xbig[:, TC * k:TC * (k + 1)],
                                start=(k == 0), stop=(k == 15))
                    return f

                def kv_piece(half):
                    def f():
                        if "kv" not in hold:
                            hold["kv"] = pss.tile([128, 2 * TC], fp32,
                                                  tag="pss", name=f"kv{tc8}")
                        for k in range(8 * half, 8 * half + 8):
                            nc.tensor.matmul(
                                hold["kv"][:, 0:TC],
                                wqkv[:, 384 * k + 256:384 * (k + 1)],
                                xbig[:, TC * k:TC * (k + 1)],
                                start=(k == 0), stop=(k == 15))
                    return f

                def kv_cons():
                    psKV = hold["kv"]
                    ctk = trgt[0:64, 2 * TC:3 * TC]
                    stk = trgt[64:128, 2 * TC:3 * TC]
                    rope(k2[0:64, tsl], psKV[:, 0:TC], ctk, stk, 64)
                    nc.gpsimd.tensor_copy(k2[64:128, tsl], k2[0:64, tsl])
                    vt = work.tile([64, TC], bf16, tag="vt", name="vt")
                    nc.vector.tensor_copy(vt[:], psKV[64:128, 0:TC])
                    vtr = work.tile([128, 256], bf16, tag="vtr", name="vtr")
                    for j in range(4):
                        nc.sync.dma_start_transpose(
                            vtr[:, 64 * j:64 * (j + 1)],
                            vt[:, 128 * j:128 * (j + 1)])
                    kb0 = 4 * tc8
                    nc.vector.tensor_copy(
                        vatt[:, 65 * kb0:65 * (kb0 + 4)].rearrange(
                            "p (n w) -> p n w", n=4)[:, :, 0:64],
                        vtr[:, :].rearrange("p (n w) -> p n w", n=4))

                def rope_q():
                    ctq, stq = trgt[:, 0:TC], trgt[:, TC:2 * TC]
                    for qt in range(2):
                        rope(qT[qt][:, tsl],
                             hold["q"][:, TC * qt:TC * (qt + 1)],
                             ctq, stq, 128)

                return [q_piece(0, 0), q_piece(0, 1), q_piece(1, 0),
                        q_piece(1, 1), kv_piece(0), kv_piece(1), kv_cons,
                        rope_q]

            prev = None
            for tc8 in range(NTC):
                b, cq = divmod(tc8, 4)
                xbig = xs.tile([128, 16 * TC], bf16, tag="x", name=f"x{tc8}")
                nc.sync.dma_start(xbig[:], xTb[tc8])
                trgt = trg.tile([128, 3 * TC], fp32, tag="t", name=f"trig{tc8}")
                nc.sync.dma_start(trgt[:], trig_d[tc8])
                if tc8 == NTC - 1:
                    nc.gpsimd.dma_start(wop[0][:], wob_d[0])
                    nc.gpsimd.dma_start(wop[1][:], wob_d[1])

                steps0 = attn_pair_steps(0, *prev) if prev else []
                pieces = build_pieces(tc8, xbig, trgt)
                for i in range(max(len(steps0), len(pieces))):
                    if i < len(steps0):
                        steps0[i]()
                    if i < len(pieces):
                        pieces[i]()
                if prev is not None:
                    attn_pair(1, *prev)
                prev = (b, cq)

            # final chunk: pair 1 first; its a2a + normalize overlap pair 0's
            # attention tail and collective
            ao = {}

            def norm_ao(t):
                den_sb = work.tile([2 * NC, TL], fp32, tag="den", name=f"den{t}")
                # raster orders match: src (r, 2h+i, :) <-> dest row 2r+h
                nc.gpsimd.dma_start(den_sb[:, :].bitcast(bf16),
                                    a2a_out[t][:, 128:132, :])
                rall = work.tile([2 * NC, TL], fp32, tag="rall", name=f"rall{t}")
                nc.vector.reciprocal_approx_fast(rall[:], den_sb[:])
                rstage = dram.tile([2 * NC, TL], fp32, tag=f"rstage{t}",
                                   name=f"rstage{t}")
                nc.sync.dma_start(rstage[:], rall[:])
                engs = [nc.scalar, nc.sync, nc.gpsimd]
                for r in range(NC):
                    kk = 2 * r + t
                    tl_ = aopool.tile([128, TL], bf16, tag=f"ao{kk}",
                                      name=f"ao{kk}")
                    engs[r % 3].dma_start(tl_[:], a2a_out[t][r, 0:128, :])
                    rb2 = work.tile([128, TL], fp32, tag="rb2", name="rb2")
                    engs[(r + 1) % 3].dma_start(
                        rb2[0:64, :],
                        rstage[2 * r:2 * r + 1, :].broadcast_to([64, TL]))
                    engs[(r + 2) % 3].dma_start(
                        rb2[64:128, :],
                        rstage[2 * r + 1:2 * r + 2, :].broadcast_to([64, TL]))
                    nc.vector.tensor_mul(tl_[:], tl_[:], rb2[:])
                    ao[kk] = tl_

            attn_pair(1, *prev)
            fire_a2a(1)
            # wo halves 2/3 into the (now free) x-stream buffers
            wos = {0: wop[0], 1: wop[1]}
            for nt in (2, 3):
                wos[nt] = xs.tile([128, 16 * TC], bf16, tag="x", name=f"wo{nt}")
                nc.gpsimd.dma_start(wos[nt][:], wob_d[nt])
            attn_pair(0, *prev)
            norm_ao(1)
            fire_a2a(0)

            # ====== Phase 2 (split-K): pair-1 contributions run during the
            # pair-0 collective; pair-0 pass closes the sum ======
            parts = {}
            for nt in range(4):
                for tt in range(TL // 128):
                    ps = po.tile([128, TC], fp32, tag="po", name=f"p1_{nt}_{tt}")
                    for j, kk in enumerate(range(1, 16, 2)):
                        nc.tensor.matmul(ps[:],
                                         ao[kk][:, 128 * tt:128 * (tt + 1)],
                                         wos[nt][:, TC * kk:TC * (kk + 1)],
                                         start=(j == 0), stop=(j == 7))
                    pt = aopool.tile([128, TC], bf16, tag=f"pt{nt}_{tt}",
                                     name=f"pt{nt}_{tt}")
                    nc.vector.tensor_copy(pt[:], ps[:])
                    parts[(nt, tt)] = pt

            norm_ao(0)
            for nt in range(4):
                for tt in range(TL // 128):
                    ps = po.tile([128, TC], fp32, tag="po", name=f"p0_{nt}_{tt}")
                    for j, kk in enumerate(range(0, 16, 2)):
                        nc.tensor.matmul(ps[:],
                                         ao[kk][:, 128 * tt:128 * (tt + 1)],
                                         wos[nt][:, TC * kk:TC * (kk + 1)],
                                         start=(j == 0), stop=(j == 7))
                    ob = work.tile([128, TC], fp32, tag="ob", name="ob")
                    nc.vector.tensor_add(ob[:], ps[:], parts[(nt, tt)][:])
                    nc.gpsimd.dma_start(out_d[128 * tt:128 * (tt + 1),
                                              TC * nt:TC * (nt + 1)], ob[:])

    nc.compile()
    return nc


def _prep_inputs(x, cos, sin, wq, wk, wv, wo):
    x = np.asarray(x, F32)
    cos = np.asarray(cos, F32)
    sin = np.asarray(sin, F32)
    wq = np.asarray(wq, F32)
    wk = np.asarray(wk, F32)
    wv = np.asarray(wv, F32)
    wo = np.asarray(wo, F32)

    xT = np.ascontiguousarray(x.reshape(T, HID).T).astype(BF16)      # [HID, T]
    # tile-packed: [c, p, 512k+t] = xT[128k+p, 512c+t]
    xTb = np.ascontiguousarray(
        xT.reshape(16, 128, NTC, TC).transpose(2, 1, 0, 3)
        .reshape(NTC, 128, 16 * TC))

    pos = np.arange(T) % S
    sign = np.concatenate([-np.ones(D // 2, F32), np.ones(D // 2, F32)])
    ctk = np.ascontiguousarray(cos[pos].T)                      # [64, T]
    stk = np.ascontiguousarray((sin[pos] * sign).T)             # [64, T]
    ctq = np.concatenate([ctk, ctk], 0) * F32(1.0 / np.sqrt(D))  # [128, T]
    stq = np.concatenate([stk, stk], 0) * F32(1.0 / np.sqrt(D))
    trig = np.zeros((NTC, 128, 3 * TC), F32)
    for c in range(NTC):
        sl = slice(TC * c, TC * (c + 1))
        trig[c, :, 0:TC] = ctq[:, sl]
        trig[c, :, TC:2 * TC] = stq[:, sl]
        trig[c, 0:64, 2 * TC:3 * TC] = ctk[:, sl]
        trig[c, 64:128, 2 * TC:3 * TC] = stk[:, sl]

    kl = np.arange(KB)
    tri1 = (kl[None, :] >= kl[:, None]).astype(BF16)      # [128 key, 128 q]
    tri = np.ascontiguousarray(np.concatenate([tri1] * 4, axis=1))

    wob = np.ascontiguousarray(
        wo.astype(BF16).reshape(16, 128, 4, TC).transpose(2, 1, 0, 3)
        .reshape(4, 128, 16 * TC))

    in_maps = []
    for c in range(NC):
        wq_c = np.ascontiguousarray(
            wq[:, c * LH * D:(c + 1) * LH * D]).astype(BF16)
        wkv_c = np.concatenate(
            [wk[:, c * D:(c + 1) * D], wv[:, c * D:(c + 1) * D]], 1).astype(BF16)
        wqkv = np.ascontiguousarray(np.concatenate(
            [wq_c.reshape(16, 128, 256), wkv_c.reshape(16, 128, 128)],
            axis=2).transpose(1, 0, 2).reshape(128, 16 * 384))
        in_maps.append({
            "xTb": xTb, "trig": trig, "wqkv": wqkv, "wob": wob, "tri": tri,
        })
    return in_maps


def get_nc():
    if "nc" not in _CACHE:
        _CACHE["nc"] = _build()
    return _CACHE["nc"]


def run(in_maps, **kwargs):
    nc = get_nc()
    return run_bass_kernel_spmd(nc, in_maps, core_ids=list(range(NC)), **kwargs)


def kernel(x, cos, sin, wq, wk, wv, wo):
    in_maps = _prep_inputs(x, cos, sin, wq, wk, wv, wo)
    res = run(in_maps)
    out = np.empty((T, HID), F32)
    for c in range(NC):
        out[TL * c:TL * (c + 1)] = res.results[c]["out"]
    return out.reshape(B, S, HID)


# revision 21
# speedup vs baseline: 1.0626x; 1.0626x over previous
"""Distributed Bass kernel for nn_Attention (B=2, S=2048, HID=2048, H=32, KVH=8, D=64).

Sharding (8 NeuronCores, uniform SPMD):
  - Head-parallel attention: core c owns kv-head c + its 4 GQA query heads.
    x replicated (host-transposed, tile-packed bf16); per-core Q^T [256,4096],
    K^T [64,4096] (+RoPE, 1/sqrt(D) folded into the Q trig tables) and
    V [tok,64] per key block with a shared ones-column for the softmax
    denominator.
  - Phase-1 chunks and attention are software-pipelined with a one-chunk
    lag: chunk N's projection matmuls are interleaved between chunk N-1's
    attention units so the PE stays dense while ACT (exp — the attention
    bottleneck) drains, and the HAM clock gate stays released.
  - Attention processes a head PAIR per unit: the two heads' S^T matmuls
    sit on disjoint PE row groups back-to-back, one exp covers both, and
    the AV matmuls share the stationary V block.
  - Causal handling: S^T/AV matmuls are ragged on diagonal key-blocks
    (fully-masked query ranges skipped); only the 128x128 triangle is
    multiplied by a mask.
  - One AllToAll per head-pair re-shards attn^T to token-parallel; raw f32
    denominators ride in the same buffer as four bf16-bitcast rows.
    The final chunk runs pair 1 first so its collective hides under
    pair 0's attention tail.
  - Phase 2 (token-parallel): out rows = attn^T.T @ wo, wo prefetched
    during attention.

All large inputs are host-prepacked so each transfer is a contiguous
block (full-rate DMA descriptors).  PE warmup matmuls at t=0 release the
HAM clock gate before real work arrives.
"""

import numpy as np
import ml_dtypes

import concourse.bass as bass
import concourse.mybir as mybir
import concourse.tile as tile
from concourse import bacc
from concourse.bass_utils import run_bass_kernel_spmd

BF16 = ml_dtypes.bfloat16
F32 = np.float32

B, S, HID = 2, 2048, 2048
H, KVH, D = 32, 8, 64
NC = 8
T = B * S              # 4096 flat tokens
TL = T // NC           # 512 tokens per core (phase-2 output rows)
LH = H // NC           # 4 local q-heads
KB = 128               # key block
TC = 512               # phase-1 token chunk / attention query chunk
NTC = T // TC          # 8 chunks

_CACHE = {}


def _build():
    fp32 = mybir.dt.float32
    bf16 = mybir.dt.bfloat16

    nc = bacc.Bacc("TRN2", target_bir_lowering=False, debug=False, num_devices=NC)

    xTb = nc.dram_tensor("xTb", [NTC, 128, 16 * TC], bf16, kind="ExternalInput")
    trig_d = nc.dram_tensor("trig", [NTC, 128, 3 * TC], fp32, kind="ExternalInput")
    wqkv_d = nc.dram_tensor("wqkv", [128, 16 * 384], bf16, kind="ExternalInput")
    wob_d = nc.dram_tensor("wob", [4, 128, 16 * TC], bf16, kind="ExternalInput")
    tri_d = nc.dram_tensor("tri", [128, 4 * KB], bf16, kind="ExternalInput")
    out_d = nc.dram_tensor("out", [TL, HID], fp32, kind="ExternalOutput")

    with tile.TileContext(nc) as tc:
        with (
            tc.tile_pool(name="persist", bufs=1) as persist,
            tc.tile_pool(name="xs", bufs=2) as xs,
            tc.tile_pool(name="trg", bufs=2) as trg,
            tc.tile_pool(name="work", bufs=2) as work,
            tc.tile_pool(name="expool", bufs=4) as expool,
            tc.tile_pool(name="aop", bufs=1) as aopool,
            tc.tile_pool(name="projq", bufs=1, space="PSUM") as projq,
            tc.tile_pool(name="pss", bufs=2, space="PSUM") as pss,
            tc.tile_pool(name="po", bufs=2, space="PSUM") as po,
            tc.tile_pool(name="dram", bufs=1, space="DRAM") as dram,
        ):
            # ---- persistent tiles ----
            qT = [persist.tile([128, T], bf16, tag=f"qT{t}", name=f"qT{t}")
                  for t in range(2)]
            k2 = persist.tile([128, T], bf16, tag="k2", name="k2")
            vatt = persist.tile([128, (T // KB) * 65], bf16, tag="vatt", name="vatt")
            attnT = [persist.tile([128, T], bf16, tag=f"attnT{t}", name=f"attnT{t}")
                     for t in range(2)]
            tri = persist.tile([128, 4 * KB], bf16, tag="tri", name="tri")
            wqkv = persist.tile([128, 16 * 384], bf16, tag="wqkv", name="wqkv")
            wop = [persist.tile([128, 16 * TC], bf16, tag=f"wop{t}", name=f"wop{t}")
                   for t in range(2)]
            ident = persist.tile([128, 128], bf16, tag="ident", name="ident")

            nc.sync.dma_start(wqkv[:], wqkv_d[:])
            nc.sync.dma_start(tri[:], tri_d[:])
            from concourse.masks import make_identity
            make_identity(nc, ident[:])
            # ones-columns of vatt (65th col of each of the 32 key blocks)
            nc.gpsimd.memset(
                vatt[:, :].rearrange("p (n w) -> p n w", n=T // KB)[:, :, 64:65],
                1.0)

            # ---- PE warmup: release the HAM clock gate during input DMA ----
            wps = po.tile([128, TC], fp32, tag="po", name="warm")
            for _ in range(40):
                nc.tensor.matmul(wps[:], ident[:], tri[:],
                                 start=True, stop=True)

            # ---- collectives staging (DRAM) ----
            # rows 0:128 = attn^T, rows 128:132 = two raw f32 denominator
            # rows bit-cast to four bf16 rows (AllToAll is bypass).
            a2a_in = [dram.tile([NC, 132, TL], bf16, tag=f"a2a_in{t}",
                                name=f"a2a_in{t}") for t in range(2)]
            a2a_out = [dram.tile([NC, 132, TL], bf16, tag=f"a2a_out{t}",
                                 name=f"a2a_out{t}") for t in range(2)]

            def rope(out_ap, ps, ct, st, npart):
                """out = ps*ct + swap32(ps)*st (st carries the rotate-half sign)."""
                t1 = work.tile([128, TC], fp32, tag="rope_t1", name="t1")
                t2 = work.tile([128, TC], fp32, tag="rope_t2", name="t2")
                nc.vector.tensor_mul(t1[:npart, :], ps[:npart, :], ct[:npart, :])
                for base in range(0, npart, 64):
                    a, b2 = base, base + 32
                    nc.vector.tensor_mul(t2[a:a + 32, :], ps[b2:b2 + 32, :],
                                         st[a:a + 32, :])
                    nc.vector.tensor_mul(t2[b2:b2 + 32, :], ps[a:a + 32, :],
                                         st[b2:b2 + 32, :])
                nc.vector.tensor_add(out_ap, t1[:npart, :], t2[:npart, :])

            def attn_pair_steps(pair, b, cq):
                """Causal attention for one head pair over a 512-query chunk,
                as a list of per-key-block emission steps (so projection
                matmul pieces can interleave and keep ACT fed).  Each step k
                emits S^T(k)+exp(k)+mask(k) and AV(k-1); a final step emits
                the last AV and the drains."""
                qtile = qT[pair]
                nkb = 4 * (cq + 1)
                qs = S * b + TC * cq
                st = {}

                def stile(kbi):
                    d = kbi - (nkb - 4)   # >=0 on diagonal key-blocks
                    qo = 128 * d if d >= 0 else 0
                    kpos = S * b + KB * kbi
                    psS = pss.tile([128, 2 * TC], fp32, tag="pss",
                                   name=f"psS{pair}_{b}_{cq}_{kbi}")
                    for h in range(2):
                        nc.tensor.matmul(
                            psS[:, TC * h + qo:TC * (h + 1)],
                            k2[64 * h:64 * h + 64, kpos:kpos + KB],
                            qtile[64 * h:64 * h + 64, qs + qo:qs + TC],
                            start=True, stop=True)
                    ex = expool.tile([128, 2 * TC + 3 * KB], bf16, tag="ex",
                                     name="ex")
                    nc.scalar.activation(ex[:, 0:2 * TC], psS[:],
                                         mybir.ActivationFunctionType.Exp)
                    if d >= 0:
                        # 128x128 triangle mask (both head segments, one
                        # strided op) on gpsimd — keeps it off the DVE FIFO
                        exm = ex[:, qo:qo + 2 * TC].rearrange(
                            "p (n w) -> p n w", n=2)[:, :, 0:KB]
                        nc.gpsimd.tensor_mul(
                            exm, exm,
                            tri[:, 0:2 * KB].rearrange("p (n w) -> p n w", n=2))
                    st[kbi] = (ex, qo)

                def av(kbi):
                    ex, qo = st.pop(kbi)
                    vb = vatt[:, 65 * (16 * b + kbi):65 * (16 * b + kbi) + 65]
                    for h in range(2):
                        nc.tensor.matmul(
                            st["psO"][h][0:65, qo:TC], vb,
                            ex[:, TC * h + qo:TC * (h + 1)],
                            start=(kbi == 0), stop=(kbi == nkb - 1),
                            skip_group_check=True)

                def first():
                    st["psO"] = [po.tile([128, TC], fp32, tag="po",
                                         name=f"psO{pair}{h}_{b}_{cq}")
                                 for h in range(2)]
                    stile(0)

                def mid(kbi):
                    stile(kbi)
                    av(kbi - 1)

                def last():
                    av(nkb - 1)
                    jj = 4 * b + cq
                    psO = st.pop("psO")
                    for h in range(2):
                        nc.vector.tensor_copy(
                            attnT[pair][64 * h:64 * h + 64, qs:qs + TC],
                            psO[h][0:D, :])
                        ds = work.tile([1, TC], fp32, tag="ds", name="ds")
                        nc.vector.tensor_copy(ds[:], psO[h][D:D + 1, :])
                        nc.gpsimd.dma_start(
                            a2a_in[pair][jj, 128 + 2 * h:130 + 2 * h, :],
                            ds[:].bitcast(bf16))
                    nc.sync.dma_start(a2a_in[pair][jj, 0:128, :],
                                      attnT[pair][:, TC * jj:TC * (jj + 1)])

                return ([first] + [(lambda k: lambda: mid(k))(k)
                                   for k in range(1, nkb)] + [last])

            def attn_pair(pair, b, cq):
                for s in attn_pair_steps(pair, b, cq):
                    s()

            def fire_a2a(t):
                nc.gpsimd.collective_compute(
                    "AllToAll", mybir.AluOpType.bypass,
                    replica_groups=[list(range(NC))],
                    ins=[a2a_in[t].opt()], outs=[a2a_out[t].opt()])

            # ========== pipelined phase 1 + attention (one-chunk lag) ==========
            # Chunk N's projection matmuls are emitted in 8-matmul pieces
            # interleaved between chunk N-1's pair-0 attention steps, so the
            # PE queue never has a long projection block starving ACT.
            def build_pieces(tc8, xbig, trgt):
                tsl = slice(TC * tc8, TC * (tc8 + 1))
                hold = {}

                def q_piece(qt, half):
                    def f():
                        if "q" not in hold:
                            hold["q"] = projq.tile([128, 2 * TC], fp32,
                                                   tag="pq", name=f"pq{tc8}")
                        for k in range(8 * half, 8 * half + 8):
                            nc.tensor.matmul(
                                hold["q"][:, TC * qt:TC * (qt + 1)],
                                wqkv[:, 384 * k + 128 * qt:
                                     384 * k + 128 * (qt + 1)],
                                xbig[:, TC * k:TC * (k + 1)],
                                start=(k == 0), stop=(k == 15))
                    return f

                def kv_piece(half):
                    def f():
                        if "kv" not in hold:
                            hold["kv"] = pss.tile([128, 2 * TC], fp32,
                                                  tag="pss", name=f"kv{tc8}")
                        for k in range(8 * half, 8 * half + 8):
                            nc.tensor.matmul(
                                hold["kv"][:, 0:TC],
                                wqkv[:, 384 * k + 256:384 * (k + 1)],
                                xbig[:, TC * k:TC * (k + 1)],
                                start=(k == 0), stop=(k == 15))
                    return f

                def kv_cons():
                    psKV = hold["kv"]
                    ctk = trgt[0:64, 2 * TC:3 * TC]
                    stk = trgt[64:128, 2 * TC:3 * TC]
                    rope(k2[0:64, tsl], psKV[:, 0:TC], ctk, stk, 64)
                    nc.gpsimd.tensor_copy(k2[64:128, tsl], k2[0:64, tsl])
                    vt = work.tile([64, TC], bf16, tag="vt", name="vt")
                    nc.vector.tensor_copy(vt[:], psKV[64:128, 0:TC])
                    vtr = work.tile([128, 256], bf16, tag="vtr", name="vtr")
                    for j in range(4):
                        nc.sync.dma_start_transpose(
                            vtr[:, 64 * j:64 * (j + 1)],
                            vt[:, 128 * j:128 * (j + 1)])
                    kb0 = 4 * tc8
                    nc.vector.tensor_copy(
                        vatt[:, 65 * kb0:65 * (kb0 + 4)].rearrange(
                            "p (n w) -> p n w", n=4)[:, :, 0:64],
                        vtr[:, :].rearrange("p (n w) -> p n w", n=4))

                def rope_q(qt):
                    def f():
                        ctq, stq = trgt[:, 0:TC], trgt[:, TC:2 * TC]
                        rope(qT[qt][:, tsl],
                             hold["q"][:, TC * qt:TC * (qt + 1)],
                             ctq, stq, 128)
                    return f

                return [q_piece(0, 0), q_piece(0, 1), rope_q(0),
                        q_piece(1, 0), q_piece(1, 1), rope_q(1),
                        kv_piece(0), kv_piece(1), kv_cons]

            prev = None
            for tc8 in range(NTC):
                b, cq = divmod(tc8, 4)
                xbig = xs.tile([128, 16 * TC], bf16, tag="x", name=f"x{tc8}")
                nc.sync.dma_start(xbig[:], xTb[tc8])
                trgt = trg.tile([128, 3 * TC], fp32, tag="t", name=f"trig{tc8}")
                nc.sync.dma_start(trgt[:], trig_d[tc8])
                if tc8 == NTC - 1:
                    nc.gpsimd.dma_start(wop[0][:], wob_d[0])
                    nc.gpsimd.dma_start(wop[1][:], wob_d[1])

                steps0 = attn_pair_steps(0, *prev) if prev else []
                pieces = build_pieces(tc8, xbig, trgt)
                for i in range(max(len(steps0), len(pieces))):
                    if i < len(steps0):
                        steps0[i]()
                    if i < len(pieces):
                        pieces[i]()
                if prev is not None:
                    attn_pair(1, *prev)
                prev = (b, cq)

            # final chunk: pair 1 first; its a2a + normalize overlap pair 0's
            # attention tail and collective
            ao = {}

            def norm_ao(t):
                den_sb = work.tile([2 * NC, TL], fp32, tag="den", name=f"den{t}")
                # raster orders match: src (r, 2h+i, :) <-> dest row 2r+h
                nc.gpsimd.dma_start(den_sb[:, :].bitcast(bf16),
                                    a2a_out[t][:, 128:132, :])
                rall = work.tile([2 * NC, TL], fp32, tag="rall", name=f"rall{t}")
                nc.vector.reciprocal_approx_fast(rall[:], den_sb[:])
                rstage = dram.tile([2 * NC, TL], fp32, tag=f"rstage{t}",
                                   name=f"rstage{t}")
                nc.sync.dma_start(rstage[:], rall[:])
                engs = [nc.scalar, nc.sync, nc.gpsimd]
                for r in range(NC):
                    kk = 2 * r + t
                    tl_ = aopool.tile([128, TL], bf16, tag=f"ao{kk}",
                                      name=f"ao{kk}")
                    engs[r % 3].dma_start(tl_[:], a2a_out[t][r, 0:128, :])
                    rb2 = work.tile([128, TL], fp32, tag="rb2", name="rb2")
                    engs[(r + 1) % 3].dma_start(
                        rb2[0:64, :],
                        rstage[2 * r:2 * r + 1, :].broadcast_to([64, TL]))
                    engs[(r + 2) % 3].dma_start(
                        rb2[64:128, :],
                        rstage[2 * r + 1:2 * r + 2, :].broadcast_to([64, TL]))
                    nc.vector.tensor_mul(tl_[:], tl_[:], rb2[:])
                    ao[kk] = tl_

            attn_pair(1, *prev)
            fire_a2a(1)
            # wo halves 2/3 into the (now free) x-stream buffers
            wos = {0: wop[0], 1: wop[1]}
            for nt in (2, 3):
                wos[nt] = xs.tile([128, 16 * TC], bf16, tag="x", name=f"wo{nt}")
                nc.gpsimd.dma_start(wos[nt][:], wob_d[nt])
            attn_pair(0, *prev)
            norm_ao(1)
            fire_a2a(0)

            # ====== Phase 2 (split-K): pair-1 contributions run during the
            # pair-0 collective; pair-0 pass closes the sum ======
            parts = {}
            for nt in range(4):
                for tt in range(TL // 128):
                    ps = po.tile([128, TC], fp32, tag="po", name=f"p1_{nt}_{tt}")
                    for j, kk in enumerate(range(1, 16, 2)):
                        nc.tensor.matmul(ps[:],
                                         ao[kk][:, 128 * tt:128 * (tt + 1)],
                                         wos[nt][:, TC * kk:TC * (kk + 1)],
                                         start=(j == 0), stop=(j == 7))
                    pt = aopool.tile([128, TC], bf16, tag=f"pt{nt}_{tt}",
                                     name=f"pt{nt}_{tt}")
                    nc.vector.tensor_copy(pt[:], ps[:])
                    parts[(nt, tt)] = pt

            norm_ao(0)
            for nt in range(4):
                for tt in range(TL // 128):
                    ps = po.tile([128, TC], fp32, tag="po", name=f"p0_{nt}_{tt}")
                    for j, kk in enumerate(range(0, 16, 2)):
                        nc.tensor.matmul(ps[:],
                                         ao[kk][:, 128 * tt:128 * (tt + 1)],
                                         wos[nt][:, TC * kk:TC * (kk + 1)],
                                         start=(j == 0), stop=(j == 7))
                    ob = work.tile([128, TC], fp32, tag="ob", name="ob")
                    nc.vector.tensor_add(ob[:], ps[:], parts[(nt, tt)][:])
                    nc.gpsimd.dma_start(out_d[128 * tt:128 * (tt + 1),
                                              TC * nt:TC * (nt + 1)], ob[:])

    nc.compile()
    return nc


def _prep_inputs(x, cos, sin, wq, wk, wv, wo):
    x = np.asarray(x, F32)
    cos = np.asarray(cos, F32)
    sin = np.asarray(sin, F32)
    wq = np.asarray(wq, F32)
    wk = np.asarray(wk, F32)
    wv = np.asarray(wv, F32)
    wo = np.asarray(wo, F32)

    xT = np.ascontiguousarray(x.reshape(T, HID).T).astype(BF16)      # [HID, T]
    # tile-packed: [c, p, 512k+t] = xT[128k+p, 512c+t]
    xTb = np.ascontiguousarray(
        xT.reshape(16, 128, NTC, TC).transpose(2, 1, 0, 3)
        .reshape(NTC, 128, 16 * TC))

    pos = np.arange(T) % S
    sign = np.concatenate([-np.ones(D // 2, F32), np.ones(D // 2, F32)])
    ctk = np.ascontiguousarray(cos[pos].T)                      # [64, T]
    stk = np.ascontiguousarray((sin[pos] * sign).T)             # [64, T]
    ctq = np.concatenate([ctk, ctk], 0) * F32(1.0 / np.sqrt(D))  # [128, T]
    stq = np.concatenate([stk, stk], 0) * F32(1.0 / np.sqrt(D))
    trig = np.zeros((NTC, 128, 3 * TC), F32)
    for c in range(NTC):
        sl = slice(TC * c, TC * (c + 1))
        trig[c, :, 0:TC] = ctq[:, sl]
        trig[c, :, TC:2 * TC] = stq[:, sl]
        trig[c, 0:64, 2 * TC:3 * TC] = ctk[:, sl]
        trig[c, 64:128, 2 * TC:3 * TC] = stk[:, sl]

    kl = np.arange(KB)
    tri1 = (kl[None, :] >= kl[:, None]).astype(BF16)      # [128 key, 128 q]
    tri = np.ascontiguousarray(np.concatenate([tri1] * 4, axis=1))

    wob = np.ascontiguousarray(
        wo.astype(BF16).reshape(16, 128, 4, TC).transpose(2, 1, 0, 3)
        .reshape(4, 128, 16 * TC))

    in_maps = []
    for c in range(NC):
        wq_c = np.ascontiguousarray(
            wq[:, c * LH * D:(c + 1) * LH * D]).astype(BF16)
        wkv_c = np.concatenate(
            [wk[:, c * D:(c + 1) * D], wv[:, c * D:(c + 1) * D]], 1).astype(BF16)
        wqkv = np.ascontiguousarray(np.concatenate(
            [wq_c.reshape(16, 128, 256), wkv_c.reshape(16, 128, 128)],
            axis=2).transpose(1, 0, 2).reshape(128, 16 * 384))
        in_maps.append({
            "xTb": xTb, "trig": trig, "wqkv": wqkv, "wob": wob, "tri": tri,
        })
    return in_maps


def get_nc():
    if "nc" not in _CACHE:
        _CACHE["nc"] = _build()
    return _CACHE["nc"]


def run(in_maps, **kwargs):
    nc = get_nc()
    return run_bass_kernel_spmd(nc, in_maps, core_ids=list(range(NC)), **kwargs)


def kernel(x, cos, sin, wq, wk, wv, wo):
    in_maps = _prep_inputs(x, cos, sin, wq, wk, wv, wo)
    res = run(in_maps)
    out = np.empty((T, HID), F32)
    for c in range(NC):
        out[TL * c:TL * (c + 1)] = res.results[c]["out"]
    return out.reshape(B, S, HID)
